# revision 22
# baseline (speedup 1.0000x reference)
"""Trainium2 Bass kernel for nn_PlgaLayer_63196148793962.

Per-head energy-curvature tensor construction followed by causal scaled
dot-product attention, sharded (batch x head) across 8 NeuronCores.

Shapes (full): Hin/Hk/Hv [2,16,2048,64], A [2,16,64,64],
weights W/b/pw/a_vec/ba [16,64,64], mask [2,1,2048,2048] (causal),
output [2,2048,16,64].

Each core gets 2 heads x 2 batches = 4 independent (b,h) instances.

Device algorithm per instance (all matmuls fp32r on the PE):
  prep:  x = W@A + b;  AW = x^2*sigmoid(x) + eps;  Ap = exp(pw*ln(AW));
         avAp = a_vec@Ap + ba                      (64x64 matrices)
  Q^T  = avAp(lhsT) @ Hin^T                        [64, 2048], duplicated
         to both partition halves for PE row-group packing
  S^T  = Hk^T-chunk(lhsT) @ Q^T-tile               [128k, 512q] per chunk,
         even/odd chunks packed at row groups 0-63 / 64-127
  P^T  = exp(S^T / 8) * causal_tri                 (ScalarE, exp only on
         the causally live columns)
  O^T  = sum_k V'chunk(lhsT) @ P^T-chunk           V' has a ones column ->
         row 64 of O^T accumulates the softmax denominator
  out  = transpose(O^T) * 1/denom                  (PE transpose + DVE)

Causality is exploited structurally: k-chunks with k0 > q_max of the
q-tile are skipped entirely (halves attention FLOPs); the mask input is
not transferred to the device.
"""

import sys

sys.path.insert(0, "/opt/trn_rl_repo")

import numpy as np

import concourse.bass as bass
import concourse.mybir as mybir
import concourse.tile as tile
from concourse.vector_clock import ScopedClock
from concourse.bass_utils import run_bass_kernel_spmd
from concourse.masks import make_identity, make_upper_triangular

F32 = mybir.dt.float32
F32R = mybir.dt.float32r
FP16 = mybir.dt.float16
BF16 = mybir.dt.bfloat16
AF = mybir.ActivationFunctionType

B, H, S, D = 2, 16, 2048, 64
N_CORES = 8
H_PER_CORE = H // N_CORES  # 2
INST = B * H_PER_CORE      # 4 (b, h_local) instances per core
QT = 512                   # q-tile width
NQT = S // QT              # 4
NCH = S // 128             # 16 k-chunks of 128
EPS_ADJ = 1e-9

LAST_RESULTS = None  # BassKernelResults of the most recent run (for test.py)
STAGE = 3  # debug bisection: 1=loads+transposes+QT, 2=+attention full-width, 3=full


class _SplitDrainTileContext(tile.TileContext):
    """Tile's kernel-tail drain carries one sem wait per touched processor;
    the walrus build here rejects instructions with >2 sync waits. Split
    them into individual wait_ge instructions on the sync engine."""

    def _drain_and_barrier(self, tick_clock, wait_clock):
        probe = self.nc.sync.nop(nofuse=True, hint="tail_wait_probe")
        wait_clock.add_sem_waits(
            probe.ins, ScopedClock({None: tick_clock.global_clock})
        )
        si = probe.ins.sync_info
        waits = list(si.on_wait) if si and si.on_wait else []
        if len(waits) > 1:
            si.on_wait.clear()
            assert self.sems is not None
            sem_by_num = {s.num: s for s in self.sems.allocated().values()}
            for w in waits:
                self.nc.sync.wait_ge(sem_by_num[w.id], w.wait_value)
        self.nc.sync.drain()
        self.nc.all_engine_barrier()
        popped = self.nc._tile_sem_poison_stack.pop()
        assert popped is self._sem_poison
        self.nc.clear_and_free_semaphores(list(self.sems.allocated().values()))
        self.nc.all_engine_barrier()


def _r(ap):
    return ap.bitcast(F32R)


def _split_excess_waits(nc, max_waits=1):
    """walrus in this container rejects instructions carrying more than one
    sync-wait command (matmul's LDWEIGHTS struct has a single wait slot).
    Move excess waits onto same-engine NOPs inserted directly before the
    over-subscribed instruction."""
    k = 0
    for f in nc.m.functions:
        for bb in f.blocks:
            insns = bb.instructions
            idx = 0
            while idx < len(insns):
                ins = insns[idx]
                si = ins.sync_info
                waits = list(si.on_wait) if si and si.on_wait else []
                if len(waits) > max_waits:
                    keep = waits[-max_waits:]
                    move = waits[:-max_waits]
                    si.on_wait.clear()
                    for w in keep:
                        si.on_wait.append(w)
                    for j in range(0, len(move), max_waits):
                        nop = mybir.InstNoOp(name=f"waitnop-{k}", ins=[], outs=[])
                        k += 1
                        nop.engine = ins.engine
                        nsi = mybir.SyncInfo(on_wait=[], on_update=[])
                        for w in move[j : j + max_waits]:
                            nsi.on_wait.append(w)
                        nop.sync_info = nsi
                        insns.insert(idx, nop)
                        idx += 1
                idx += 1
    return nc


def _build_program():
    nc = bass.Bass()

    hin_p = nc.declare_dram_parameter("Hin", [INST, S, D], F32, isOutput=False)
    hk_p = nc.declare_dram_parameter("Hk", [INST, S, D], F32, isOutput=False)
    hv_p = nc.declare_dram_parameter("Hv", [INST, S, D], F32, isOutput=False)
    a_p = nc.declare_dram_parameter("A", [INST, D, D], F32, isOutput=False)
    wt_p = nc.declare_dram_parameter("Wt", [INST, D, D], F32, isOutput=False)
    bb_p = nc.declare_dram_parameter("bb", [INST, D, D], F32, isOutput=False)
    pw_p = nc.declare_dram_parameter("pw", [INST, D, D], F32, isOutput=False)
    avt_p = nc.declare_dram_parameter("avT", [INST, D, D], F32, isOutput=False)
    ba_p = nc.declare_dram_parameter("ba", [INST, D, D], F32, isOutput=False)
    out_p = nc.declare_dram_parameter("out", [INST, S, D], F32, isOutput=True)

    with _SplitDrainTileContext(nc) as tc:
        with (
            tc.tile_pool(name="const", bufs=1) as cpool,
            tc.tile_pool(name="smalls", bufs=1) as spool,
            tc.tile_pool(name="prep", bufs=2) as ppool,
            tc.tile_pool(name="big", bufs=3) as bpool,
            tc.tile_pool(name="pt", bufs=4) as ptpool,
            tc.tile_pool(name="eout", bufs=4) as epool,
            tc.tile_pool(name="ps_st", bufs=2, space="PSUM") as ps_st,
            tc.tile_pool(name="ps_o", bufs=2, space="PSUM") as ps_o,
            tc.tile_pool(name="ps_tr", bufs=2, space="PSUM") as ps_tr,
        ):
            ident = cpool.tile([128, 128], F32)
            make_identity(nc, ident)
            ident16 = cpool.tile([128, 128], FP16)
            make_identity(nc, ident16)
            tri16 = cpool.tile([128, 128], BF16)
            make_upper_triangular(nc, tri16, val=1.0, diag=True)

            # Small per-instance matrices, duplicated to both partition
            # halves: [128(j dup), INST, 64]
            def load_small(param, name):
                t = spool.tile([128, INST, D], F32, tag=name)
                src = param.rearrange("i j k -> j i k")
                nc.sync.dma_start(t[0:64], src)
                nc.sync.dma_start(t[64:128], src)
                return t

            a_s = load_small(a_p, "a_s")
            wt_s = load_small(wt_p, "wt_s")
            bb_s = load_small(bb_p, "bb_s")
            pw_s = load_small(pw_p, "pw_s")
            avt_s = load_small(avt_p, "avt_s")
            ba_s = load_small(ba_p, "ba_s")

            # ---- prep (per instance, interleaved into the main loop):
            # avAp for all instances, both halves ----
            avap = spool.tile([128, INST, D], FP16, tag="avap")

            def prep(i):
                x_ps = ps_tr.tile([128, 128], F32, tag="tr", name="x_ps")[:, 0:64]
                nc.tensor.matmul(x_ps[0:64], wt_s[0:64, i], a_s[0:64, i])
                nc.tensor.matmul(x_ps[64:128], wt_s[64:128, i], a_s[64:128, i])
                x_sb = ppool.tile([128, D], F32, tag="x_sb")
                nc.vector.tensor_add(x_sb, x_ps, bb_s[:, i])
                # sigmoid(x) = 1 / (1 + exp(-x)) -- via the exp table set so
                # every ACT op in the kernel shares one function set
                e_sb = ppool.tile([128, D], F32, tag="e_sb")
                nc.scalar.activation(e_sb, x_sb, AF.Exp, scale=-1.0)
                nc.vector.tensor_scalar_add(e_sb, e_sb, 1.0)
                r_sb = ppool.tile([128, D], F32, tag="r_sb")
                nc.vector.reciprocal(r_sb, e_sb)
                aw_sb = ppool.tile([128, D], F32, tag="aw_sb")
                nc.vector.tensor_mul(aw_sb, x_sb, x_sb)
                nc.vector.tensor_mul(aw_sb, aw_sb, r_sb)
                nc.vector.tensor_scalar_add(aw_sb, aw_sb, EPS_ADJ)
                ln_sb = ppool.tile([128, D], F32, tag="ln_sb")
                nc.scalar.activation(ln_sb, aw_sb, AF.Ln)
                nc.vector.tensor_mul(ln_sb, ln_sb, pw_s[:, i])
                ap_sb = ppool.tile([128, D], F32, tag="ap_sb")
                nc.scalar.activation(ap_sb, ln_sb, AF.Exp)
                av_ps = ps_tr.tile([128, 128], F32, tag="tr", name="av_ps")[:, 0:64]
                nc.tensor.matmul(av_ps[0:64], avt_s[0:64, i], ap_sb[0:64])
                nc.tensor.matmul(av_ps[64:128], avt_s[64:128, i], ap_sb[64:128])
                nc.vector.tensor_add(avap[:, i], av_ps, ba_s[:, i])

            # ---- main attention per instance ----
            for i in range(INST):
                prep(i)
                # Hin natural, paired (q, q+1024) for 128x128 PE transposes
                hin_nat = bpool.tile([128, 8, 128], FP16, tag="hin_nat")
                hin_v = hin_nat.rearrange("p t (two d) -> p t two d", two=2)
                for two in range(2):
                    nc.gpsimd.dma_start(
                        hin_v[:, :, two],
                        hin_p[i, 1024 * two : 1024 * (two + 1)].rearrange(
                            "(t p) d -> p t d", t=8, p=128
                        ),
                    )
                # Hin^T packed by q-halves: rows 0-63 d for q<1024, 64-127 for q>=1024
                hintt = bpool.tile([128, 8, 128], FP16, tag="hintt")
                for t in range(8):
                    tr16 = ps_tr.tile([128, 128], FP16, tag="tr", name="tr16")
                    nc.tensor.transpose(tr16, hin_nat[:, t], ident16)
                    nc.vector.tensor_copy(hintt[:, t], tr16)

                # Q^T duplicated to both partition halves: [128, S].
                # fp32r matmuls cannot target PSUM partition base 64, so
                # compute rows 0-63 and replicate via SBUF->SBUF DMA.
                qtd = bpool.tile([128, S], FP16, tag="qtd")
                for s in range(NQT):
                    beta = 0 if s < 2 else 64
                    cols = slice((512 * s) % 1024, (512 * s) % 1024 + 512)
                    hview = hintt.rearrange("p t c -> p (t c)")
                    qt_ps = ps_o.tile([128, QT], F32, tag="o", name="qt_ps")
                    nc.tensor.matmul(
                        qt_ps[0:64],
                        avap[beta : beta + 64, i],
                        hview[beta : beta + 64, cols],
                    )
                    nc.tensor.matmul(
                        qt_ps[64:128],
                        avap[beta : beta + 64, i],
                        hview[beta : beta + 64, cols],
                        tile_position=(beta, 64),
                    )
                    nc.vector.tensor_copy(qtd[:, 512 * s : 512 * s + 512], qt_ps)

                # Hk^T packed by chunk parity: rows 0-63 = even 128-chunks
                hk_nat = bpool.tile([128, 8, 128], FP16, tag="hk_nat")
                hk_v = hk_nat.rearrange("p t (two d) -> p t two d", two=2)
                for two in range(2):
                    nc.gpsimd.dma_start(
                        hk_v[:, :, two],
                        hk_p[i].rearrange(
                            "(t two p) d -> p t two d", t=8, two=2, p=128
                        )[:, :, two],
                    )
                ktt = bpool.tile([128, 8, 128], FP16, tag="ktt")
                for t in range(8):
                    tr16 = ps_tr.tile([128, 128], FP16, tag="tr", name="tr16")
                    nc.tensor.transpose(tr16, hk_nat[:, t], ident16)
                    nc.vector.tensor_copy(ktt[:, t], tr16)

                # V with an appended ones column (bf16): [128, chunk, 65]
                vt = bpool.tile([128, NCH, D + 1], BF16, tag="vt")
                nc.gpsimd.memset(vt, 1.0)
                nc.gpsimd.dma_start(
                    vt[:, :, 0:D],
                    hv_p[i].rearrange("(c p) d -> p c d", c=NCH, p=128),
                )

                if STAGE < 2:
                    outn0 = epool.tile([128, D], F32, tag="outn")
                    nc.vector.tensor_copy(outn0, qtd.bitcast(F32)[:, 0:D])
                    nc.sync.dma_start(out_p[i, 0:128, :], outn0)
                    continue
                for s in range(NQT):
                    o_ps = ps_o.tile([128, QT], F32, tag="o")
                    nchunks = 4 * (s + 1)
                    for p in range(nchunks // 2):
                        cpair = (2 * p, 2 * p + 1)
                        col0s = [max(0, 128 * (c - 4 * s)) for c in cpair]
                        # S^T for the even/odd chunk pair, issued back to back
                        # at PE row groups 0-63 / 64-127 so they run
                        # concurrently; one [128, 1024] PSUM pair tile.
                        pair_ps = ps_st.tile([128, 2 * QT], F32, tag="st")
                        for half, (c, col0) in enumerate(zip(cpair, col0s)):
                            beta = 64 * half
                            nc.tensor.matmul(
                                pair_ps[:, 512 * half + col0 : 512 * half + 512],
                                ktt[beta : beta + 64, p],
                                qtd[beta : beta + 64, 512 * s + col0 : 512 * s + 512],
                            )
                        # exp over the causally live columns (merged when
                        # contiguous across the pair boundary)
                        pt = ptpool.tile([128, 2 * QT], BF16, tag="pt")
                        ranges = [
                            (512 * half + col0, 512 * half + 512)
                            for half, col0 in enumerate(col0s)
                        ]
                        if ranges[0][1] == ranges[1][0]:
                            ranges = [(ranges[0][0], ranges[1][1])]
                        for lo, hi in ranges:
                            nc.scalar.activation(
                                pt[:, lo:hi], pair_ps[:, lo:hi], AF.Exp, scale=0.125
                            )
                        for half, (c, col0) in enumerate(zip(cpair, col0s)):
                            if c - 4 * s >= 0:
                                blk = 512 * half + col0
                                nc.vector.tensor_mul(
                                    pt[:, blk : blk + 128], pt[:, blk : blk + 128], tri16
                                )
                        for half, (c, col0) in enumerate(zip(cpair, col0s)):
                            nc.tensor.matmul(
                                o_ps[0:65, col0:QT],
                                vt[:, c],
                                pt[:, 512 * half + col0 : 512 * half + 512],
                                start=(c == 0),
                                stop=(c == nchunks - 1),
                            )

                    ot = epool.tile([128, QT], F32, tag="ot")
                    nc.vector.tensor_copy(ot[0:65], o_ps[0:65])
                    outn = epool.tile([128, 4, D], F32, tag="outn")
                    for j in range(4):
                        tr_ps = ps_tr.tile([128, 128], F32, tag="tr")
                        nc.tensor.transpose(
                            tr_ps[:, 0:65],
                            ot[0:65, 128 * j : 128 * j + 128],
                            ident[0:65, 0:65],
                        )
                        rec = epool.tile([128, 1], F32, tag="rec")
                        nc.vector.reciprocal(rec, tr_ps[:, 64:65])
                        nc.vector.tensor_scalar_mul(outn[:, j], tr_ps[:, 0:D], rec)
                    nc.sync.dma_start(
                        out_p[i, 512 * s : 512 * (s + 1), :].rearrange(
                            "(j p) d -> p j d", j=4, p=128
                        ),
                        outn,
                    )

    return nc


_CACHED_NC = None


def kernel(Hin, Hk, Hv, A, mask, W, b, pw, a_vec, ba):
    global _CACHED_NC, LAST_RESULTS
    if _CACHED_NC is None:
        _CACHED_NC = _split_excess_waits(_build_program())
    nc = _CACHED_NC

    Hin = np.asarray(Hin, dtype=np.float32)
    Hk = np.asarray(Hk, dtype=np.float32)
    Hv = np.asarray(Hv, dtype=np.float32)
    A = np.asarray(A, dtype=np.float32)
    W = np.asarray(W, dtype=np.float32)
    b = np.asarray(b, dtype=np.float32)
    pw = np.asarray(pw, dtype=np.float32)
    a_vec = np.asarray(a_vec, dtype=np.float32)
    ba = np.asarray(ba, dtype=np.float32)

    in_maps = []
    for c in range(N_CORES):
        hs = slice(H_PER_CORE * c, H_PER_CORE * (c + 1))

        def per_head(x, transpose=False):
            x = x[hs]  # [2, 64, 64]
            if transpose:
                x = np.swapaxes(x, 1, 2)
            return np.ascontiguousarray(np.concatenate([x, x], axis=0))  # [4,64,64]

        in_maps.append(
            {
                "Hin": np.ascontiguousarray(Hin[:, hs].reshape(INST, S, D)),
                "Hk": np.ascontiguousarray(Hk[:, hs].reshape(INST, S, D)),
                "Hv": np.ascontiguousarray(Hv[:, hs].reshape(INST, S, D)),
                "A": np.ascontiguousarray(A[:, hs].reshape(INST, D, D)),
                "Wt": per_head(W, transpose=True),
                "bb": per_head(b),
                "pw": per_head(pw),
                "avT": per_head(a_vec, transpose=True),
                "ba": per_head(ba),
            }
        )

    res = run_bass_kernel_spmd(nc, in_maps, list(range(N_CORES)))
    LAST_RESULTS = res

    out = np.empty((B, S, H, D), dtype=np.float32)
    for c in range(N_CORES):
        co = res.results[c]["out"]  # [4, 2048, 64], inst = b*2 + hl
        for bi in range(B):
            for hl in range(H_PER_CORE):
                out[bi, :, H_PER_CORE * c + hl, :] = co[bi * H_PER_CORE + hl]
    return out


# revision 23
# speedup vs baseline: 1.2744x; 1.2744x over previous
"""Trainium2 Bass kernel for nn_PlgaLayer_63196148793962.

Per-head energy-curvature tensor construction followed by causal scaled
dot-product attention, sharded (batch x head) across 8 NeuronCores.

Shapes (full): Hin/Hk/Hv [2,16,2048,64], A [2,16,64,64],
weights W/b/pw/a_vec/ba [16,64,64], mask [2,1,2048,2048] (causal),
output [2,2048,16,64].

Each core gets 2 heads x 2 batches = 4 independent (b,h) instances.

Device algorithm per instance (all matmuls fp32r on the PE):
  prep:  x = W@A + b;  AW = x^2*sigmoid(x) + eps;  Ap = exp(pw*ln(AW));
         avAp = a_vec@Ap + ba                      (64x64 matrices)
  Q^T  = avAp(lhsT) @ Hin^T                        [64, 2048], duplicated
         to both partition halves for PE row-group packing
  S^T  = Hk^T-chunk(lhsT) @ Q^T-tile               [128k, 512q] per chunk,
         even/odd chunks packed at row groups 0-63 / 64-127
  P^T  = exp(S^T / 8) * causal_tri                 (ScalarE, exp only on
         the causally live columns)
  O^T  = sum_k V'chunk(lhsT) @ P^T-chunk           V' has a ones column ->
         row 64 of O^T accumulates the softmax denominator
  out  = transpose(O^T) * 1/denom                  (PE transpose + DVE)

Causality is exploited structurally: k-chunks with k0 > q_max of the
q-tile are skipped entirely (halves attention FLOPs); the mask input is
not transferred to the device.
"""

import sys

sys.path.insert(0, "/opt/trn_rl_repo")

import numpy as np

import concourse.bass as bass
import concourse.mybir as mybir
import concourse.tile as tile
from concourse.vector_clock import ScopedClock
from concourse.bass_utils import run_bass_kernel_spmd
from concourse.masks import make_identity, make_upper_triangular

F32 = mybir.dt.float32
F32R = mybir.dt.float32r
FP16 = mybir.dt.float16
BF16 = mybir.dt.bfloat16
AF = mybir.ActivationFunctionType

B, H, S, D = 2, 16, 2048, 64
N_CORES = 8
H_PER_CORE = H // N_CORES  # 2
INST = B * H_PER_CORE      # 4 (b, h_local) instances per core
QT = 512                   # q-tile width
NQT = S // QT              # 4
NCH = S // 128             # 16 k-chunks of 128
EPS_ADJ = 1e-9

LAST_RESULTS = None  # BassKernelResults of the most recent run (for test.py)
STAGE = 3  # debug bisection: 1=loads+transposes+QT, 2=+attention full-width, 3=full


class _SplitDrainTileContext(tile.TileContext):
    """Tile's kernel-tail drain carries one sem wait per touched processor;
    the walrus build here rejects instructions with >2 sync waits. Split
    them into individual wait_ge instructions on the sync engine."""

    def _drain_and_barrier(self, tick_clock, wait_clock):
        probe = self.nc.sync.nop(nofuse=True, hint="tail_wait_probe")
        wait_clock.add_sem_waits(
            probe.ins, ScopedClock({None: tick_clock.global_clock})
        )
        si = probe.ins.sync_info
        waits = list(si.on_wait) if si and si.on_wait else []
        if len(waits) > 1:
            si.on_wait.clear()
            assert self.sems is not None
            sem_by_num = {s.num: s for s in self.sems.allocated().values()}
            for w in waits:
                self.nc.sync.wait_ge(sem_by_num[w.id], w.wait_value)
        self.nc.sync.drain()
        self.nc.all_engine_barrier()
        popped = self.nc._tile_sem_poison_stack.pop()
        assert popped is self._sem_poison
        self.nc.clear_and_free_semaphores(list(self.sems.allocated().values()))
        self.nc.all_engine_barrier()


def _r(ap):
    return ap.bitcast(F32R)


def _split_excess_waits(nc, max_waits=1):
    """walrus in this container rejects instructions carrying more than one
    sync-wait command (matmul's LDWEIGHTS struct has a single wait slot).
    Move excess waits onto same-engine NOPs inserted directly before the
    over-subscribed instruction."""
    k = 0
    for f in nc.m.functions:
        for bb in f.blocks:
            insns = bb.instructions
            idx = 0
            while idx < len(insns):
                ins = insns[idx]
                si = ins.sync_info
                waits = list(si.on_wait) if si and si.on_wait else []
                if len(waits) > max_waits:
                    keep = waits[-max_waits:]
                    move = waits[:-max_waits]
                    si.on_wait.clear()
                    for w in keep:
                        si.on_wait.append(w)
                    for j in range(0, len(move), max_waits):
                        nop = mybir.InstNoOp(name=f"waitnop-{k}", ins=[], outs=[])
                        k += 1
                        nop.engine = ins.engine
                        nsi = mybir.SyncInfo(on_wait=[], on_update=[])
                        for w in move[j : j + max_waits]:
                            nsi.on_wait.append(w)
                        nop.sync_info = nsi
                        insns.insert(idx, nop)
                        idx += 1
                idx += 1
    return nc


def _build_program():
    nc = bass.Bass()

    hin_p = nc.declare_dram_parameter("Hin", [INST, S, D], F32, isOutput=False)
    hk_p = nc.declare_dram_parameter("Hk", [INST, S, D], F32, isOutput=False)
    hv_p = nc.declare_dram_parameter("Hv", [INST, S, D], F32, isOutput=False)
    a_p = nc.declare_dram_parameter("A", [INST, D, D], F32, isOutput=False)
    wt_p = nc.declare_dram_parameter("Wt", [INST, D, D], F32, isOutput=False)
    bb_p = nc.declare_dram_parameter("bb", [INST, D, D], F32, isOutput=False)
    pw_p = nc.declare_dram_parameter("pw", [INST, D, D], F32, isOutput=False)
    avt_p = nc.declare_dram_parameter("avT", [INST, D, D], F32, isOutput=False)
    ba_p = nc.declare_dram_parameter("ba", [INST, D, D], F32, isOutput=False)
    out_p = nc.declare_dram_parameter("out", [INST, S, D], F32, isOutput=True)

    with _SplitDrainTileContext(nc) as tc:
        with (
            tc.tile_pool(name="const", bufs=1) as cpool,
            tc.tile_pool(name="smalls", bufs=1) as spool,
            tc.tile_pool(name="prep", bufs=2) as ppool,
            tc.tile_pool(name="big", bufs=3) as bpool,
            tc.tile_pool(name="pt", bufs=6) as ptpool,
            tc.tile_pool(name="eout", bufs=6) as epool,
            tc.tile_pool(name="ps_st", bufs=2, space="PSUM") as ps_st,
            tc.tile_pool(name="ps_o", bufs=2, space="PSUM") as ps_o,
            tc.tile_pool(name="ps_tr", bufs=2, space="PSUM") as ps_tr,
        ):
            ident = cpool.tile([128, 128], F32)
            make_identity(nc, ident)
            ident16 = cpool.tile([128, 128], FP16)
            make_identity(nc, ident16)
            tri16 = cpool.tile([128, 128], BF16)
            make_upper_triangular(nc, tri16, val=1.0, diag=True)

            # Small per-instance matrices, duplicated to both partition
            # halves: [128(j dup), INST, 64]
            def load_small(param, name):
                t = spool.tile([128, INST, D], F32, tag=name)
                src = param.rearrange("i j k -> j i k")
                nc.sync.dma_start(t[0:64], src)
                nc.sync.dma_start(t[64:128], src)
                return t

            a_s = load_small(a_p, "a_s")
            wt_s = load_small(wt_p, "wt_s")
            bb_s = load_small(bb_p, "bb_s")
            pw_s = load_small(pw_p, "pw_s")
            avt_s = load_small(avt_p, "avt_s")
            ba_s = load_small(ba_p, "ba_s")

            # ---- prep (per instance, interleaved into the main loop):
            # avAp for all instances, both halves ----
            avap = spool.tile([128, INST, D], FP16, tag="avap")

            def prep(i):
                x_ps = ps_tr.tile([128, 128], F32, tag="tr", name="x_ps")[:, 0:64]
                nc.tensor.matmul(x_ps[0:64], wt_s[0:64, i], a_s[0:64, i])
                nc.tensor.matmul(x_ps[64:128], wt_s[64:128, i], a_s[64:128, i])
                x_sb = ppool.tile([128, D], F32, tag="x_sb")
                nc.vector.tensor_add(x_sb, x_ps, bb_s[:, i])
                # sigmoid(x) = 1 / (1 + exp(-x)) -- via the exp table set so
                # every ACT op in the kernel shares one function set
                e_sb = ppool.tile([128, D], F32, tag="e_sb")
                nc.scalar.activation(e_sb, x_sb, AF.Exp, scale=-1.0)
                nc.vector.tensor_scalar_add(e_sb, e_sb, 1.0)
                r_sb = ppool.tile([128, D], F32, tag="r_sb")
                nc.vector.reciprocal(r_sb, e_sb)
                aw_sb = ppool.tile([128, D], F32, tag="aw_sb")
                nc.vector.tensor_mul(aw_sb, x_sb, x_sb)
                nc.vector.tensor_mul(aw_sb, aw_sb, r_sb)
                nc.vector.tensor_scalar_add(aw_sb, aw_sb, EPS_ADJ)
                ln_sb = ppool.tile([128, D], F32, tag="ln_sb")
                nc.scalar.activation(ln_sb, aw_sb, AF.Ln)
                nc.vector.tensor_mul(ln_sb, ln_sb, pw_s[:, i])
                ap_sb = ppool.tile([128, D], F32, tag="ap_sb")
                nc.scalar.activation(ap_sb, ln_sb, AF.Exp)
                av_ps = ps_tr.tile([128, 128], F32, tag="tr", name="av_ps")[:, 0:64]
                nc.tensor.matmul(av_ps[0:64], avt_s[0:64, i], ap_sb[0:64])
                nc.tensor.matmul(av_ps[64:128], avt_s[64:128, i], ap_sb[64:128])
                nc.vector.tensor_add(avap[:, i], av_ps, ba_s[:, i])

            for i in range(INST):
                prep(i)

            # ---- main attention per instance ----
            for i in range(INST):
                # Hin natural, paired (q, q+1024) for 128x128 PE transposes
                hin_nat = bpool.tile([128, 8, 128], FP16, tag="hin_nat")
                hin_v = hin_nat.rearrange("p t (two d) -> p t two d", two=2)
                for two in range(2):
                    nc.gpsimd.dma_start(
                        hin_v[:, :, two],
                        hin_p[i, 1024 * two : 1024 * (two + 1)].rearrange(
                            "(t p) d -> p t d", t=8, p=128
                        ),
                    )
                # Hin^T packed by q-halves: rows 0-63 d for q<1024, 64-127 for q>=1024
                hintt = bpool.tile([128, 8, 128], FP16, tag="hintt")
                trb = ps_st.tile([128, 8, 128], FP16, tag="st", name="trb")
                for t in range(8):
                    nc.tensor.transpose(trb[:, t], hin_nat[:, t], ident16)
                nc.vector.tensor_copy(hintt, trb)

                # Q^T duplicated to both partition halves: [128, S].
                # fp32r matmuls cannot target PSUM partition base 64, so
                # compute rows 0-63 and replicate via SBUF->SBUF DMA.
                qtd = bpool.tile([128, S], FP16, tag="qtd")
                for s in range(NQT):
                    beta = 0 if s < 2 else 64
                    cols = slice((512 * s) % 1024, (512 * s) % 1024 + 512)
                    hview = hintt.rearrange("p t c -> p (t c)")
                    qt_ps = ps_o.tile([128, QT], F32, tag="o", name="qt_ps")
                    nc.tensor.matmul(
                        qt_ps[0:64],
                        avap[beta : beta + 64, i],
                        hview[beta : beta + 64, cols],
                    )
                    nc.tensor.matmul(
                        qt_ps[64:128],
                        avap[beta : beta + 64, i],
                        hview[beta : beta + 64, cols],
                        tile_position=(beta, 64),
                    )
                    nc.vector.tensor_copy(qtd[:, 512 * s : 512 * s + 512], qt_ps)

                # Hk^T packed by chunk parity: rows 0-63 = even 128-chunks
                hk_nat = bpool.tile([128, 8, 128], FP16, tag="hk_nat")
                hk_v = hk_nat.rearrange("p t (two d) -> p t two d", two=2)
                for two in range(2):
                    nc.gpsimd.dma_start(
                        hk_v[:, :, two],
                        hk_p[i].rearrange(
                            "(t two p) d -> p t two d", t=8, two=2, p=128
                        )[:, :, two],
                    )
                ktt = bpool.tile([128, 8, 128], FP16, tag="ktt")
                trb = ps_st.tile([128, 8, 128], FP16, tag="st", name="trb")
                for t in range(8):
                    nc.tensor.transpose(trb[:, t], hk_nat[:, t], ident16)
                nc.vector.tensor_copy(ktt, trb)

                # V with an appended ones column (bf16): [128, chunk, 65]
                vt = bpool.tile([128, NCH, D + 1], BF16, tag="vt")
                nc.gpsimd.memset(vt, 1.0)
                nc.gpsimd.dma_start(
                    vt[:, :, 0:D],
                    hv_p[i].rearrange("(c p) d -> p c d", c=NCH, p=128),
                )

                if STAGE < 2:
                    outn0 = epool.tile([128, D], F32, tag="outn")
                    nc.vector.tensor_copy(outn0, qtd.bitcast(F32)[:, 0:D])
                    nc.sync.dma_start(out_p[i, 0:128, :], outn0)
                    continue
                for s in range(NQT):
                    o_ps = ps_o.tile([128, QT], F32, tag="o")
                    nchunks = 4 * (s + 1)
                    for p in range(nchunks // 2):
                        cpair = (2 * p, 2 * p + 1)
                        col0s = [max(0, 128 * (c - 4 * s)) for c in cpair]
                        # S^T for the even/odd chunk pair, issued back to back
                        # at PE row groups 0-63 / 64-127 so they run
                        # concurrently; one [128, 1024] PSUM pair tile.
                        pair_ps = ps_st.tile([128, 2 * QT], F32, tag="st")
                        for half, (c, col0) in enumerate(zip(cpair, col0s)):
                            beta = 64 * half
                            nc.tensor.matmul(
                                pair_ps[:, 512 * half + col0 : 512 * half + 512],
                                ktt[beta : beta + 64, p],
                                qtd[beta : beta + 64, 512 * s + col0 : 512 * s + 512],
                            )
                        # exp over the causally live columns (merged when
                        # contiguous across the pair boundary)
                        pt = ptpool.tile([128, 2 * QT], BF16, tag="pt")
                        ranges = [
                            (512 * half + col0, 512 * half + 512)
                            for half, col0 in enumerate(col0s)
                        ]
                        if ranges[0][1] == ranges[1][0]:
                            ranges = [(ranges[0][0], ranges[1][1])]
                        for lo, hi in ranges:
                            nc.scalar.activation(
                                pt[:, lo:hi], pair_ps[:, lo:hi], AF.Exp, scale=0.125
                            )
                        for half, (c, col0) in enumerate(zip(cpair, col0s)):
                            if c - 4 * s >= 0:
                                blk = 512 * half + col0
                                nc.vector.tensor_mul(
                                    pt[:, blk : blk + 128], pt[:, blk : blk + 128], tri16
                                )
                        for half, (c, col0) in enumerate(zip(cpair, col0s)):
                            nc.tensor.matmul(
                                o_ps[0:65, col0:QT],
                                vt[:, c],
                                pt[:, 512 * half + col0 : 512 * half + 512],
                                start=(c == 0),
                                stop=(c == nchunks - 1),
                            )

                    ot = epool.tile([128, QT], F32, tag="ot")
                    nc.vector.tensor_copy(ot[0:65], o_ps[0:65])
                    outn = epool.tile([128, 4, D], F32, tag="outn")
                    for j in range(4):
                        tr_ps = ps_tr.tile([128, 128], F32, tag="tr")
                        nc.tensor.transpose(
                            tr_ps[:, 0:65],
                            ot[0:65, 128 * j : 128 * j + 128],
                            ident[0:65, 0:65],
                        )
                        rec = epool.tile([128, 1], F32, tag="rec")
                        nc.vector.reciprocal(rec, tr_ps[:, 64:65])
                        nc.vector.tensor_scalar_mul(outn[:, j], tr_ps[:, 0:D], rec)
                    nc.sync.dma_start(
                        out_p[i, 512 * s : 512 * (s + 1), :].rearrange(
                            "(j p) d -> p j d", j=4, p=128
                        ),
                        outn,
                    )

    return nc


_CACHED_NC = None


def kernel(Hin, Hk, Hv, A, mask, W, b, pw, a_vec, ba):
    global _CACHED_NC, LAST_RESULTS
    if _CACHED_NC is None:
        _CACHED_NC = _split_excess_waits(_build_program())
    nc = _CACHED_NC

    Hin = np.asarray(Hin, dtype=np.float32)
    Hk = np.asarray(Hk, dtype=np.float32)
    Hv = np.asarray(Hv, dtype=np.float32)
    A = np.asarray(A, dtype=np.float32)
    W = np.asarray(W, dtype=np.float32)
    b = np.asarray(b, dtype=np.float32)
    pw = np.asarray(pw, dtype=np.float32)
    a_vec = np.asarray(a_vec, dtype=np.float32)
    ba = np.asarray(ba, dtype=np.float32)

    in_maps = []
    for c in range(N_CORES):
        hs = slice(H_PER_CORE * c, H_PER_CORE * (c + 1))

        def per_head(x, transpose=False):
            x = x[hs]  # [2, 64, 64]
            if transpose:
                x = np.swapaxes(x, 1, 2)
            return np.ascontiguousarray(np.concatenate([x, x], axis=0))  # [4,64,64]

        in_maps.append(
            {
                "Hin": np.ascontiguousarray(Hin[:, hs].reshape(INST, S, D)),
                "Hk": np.ascontiguousarray(Hk[:, hs].reshape(INST, S, D)),
                "Hv": np.ascontiguousarray(Hv[:, hs].reshape(INST, S, D)),
                "A": np.ascontiguousarray(A[:, hs].reshape(INST, D, D)),
                "Wt": per_head(W, transpose=True),
                "bb": per_head(b),
                "pw": per_head(pw),
                "avT": per_head(a_vec, transpose=True),
                "ba": per_head(ba),
            }
        )

    res = run_bass_kernel_spmd(nc, in_maps, list(range(N_CORES)))
    LAST_RESULTS = res

    out = np.empty((B, S, H, D), dtype=np.float32)
    for c in range(N_CORES):
        co = res.results[c]["out"]  # [4, 2048, 64], inst = b*2 + hl
        for bi in range(B):
            for hl in range(H_PER_CORE):
                out[bi, :, H_PER_CORE * c + hl, :] = co[bi * H_PER_CORE + hl]
    return out
